# revision 15
# baseline (speedup 1.0000x reference)
"""Trainium2 Bass kernel for the coverage-attention module (fp8 DoubleRow).

Math (per batch b):
    enc_feat = encoder_outputs @ W_h.T                      [S, H]
    dec_fea  = s_t_hat @ W_s.T + b_s                        [H]
    e        = tanh(enc_feat + dec_fea + coverage[:,None]*W_c[:,0])
    scores   = e @ v[0]                                     [S]
    w        = exp(scores) * mask          (softmax+mask+renorm == w/sum(w))
    attn     = w / sum(w)
    c_t      = attn @ encoder_outputs                       [H]
    coverage_new = coverage + attn

Distribution: pure data-parallel over batch, 8 batches per NeuronCore,
weights replicated.  No collectives.

Per-core dataflow:
  - enc ingest: f32 HWDGE loads split across the sync+scalar queues
    (the SWDGE cast path is ~6.6us per tile and would starve the PE),
    f32->bf16 cast split across DVE and GpSimd-Pool, xbar DMA-transpose
    to [h_p, hb, s] bf16, DVE cast *16 -> fp8.  The chain runs with a
    1-2 s-block lag inside the previous batch so no queue ever stalls.
  - W_h/W_s: f32 loads, f32 PE transposes, fused fp8*64 (resp. bf16)
    cast on the PSUM->SBUF copy.
  - main matmul in fp8 DoubleRow (2 h-blocks per instruction, 2x PE
    throughput); PSUM holds 1024*enc_feat, tanh applies scale 1/1024.
  - dec_fea + coverage*W_c folded into each PSUM group as one extra
    [2,2]-DoubleRow matmul (operands pre-scaled: ones*32, dec*32,
    cov*1, W_c*1024).
  - scores via VectorE scalar_tensor_tensor against broadcast v.
  - softmax runs incrementally per s-block: exp/mask/cast as soon as a
    block's scores exist, and c_t accumulates in PSUM (k=s matmuls on
    the bf16 nat tiles) interleaved with the next block's main matmuls.
    Only the 1/Z normalization happens at batch end -> tiny tail.
"""

import numpy as np
import ml_dtypes

import concourse.bass as bass
import concourse.tile as tile
from concourse import bacc, mybir
from concourse.bass_utils import run_bass_kernel_spmd

N_CORES = 8
B, S, H = 64, 1024, 1024
BL = B // N_CORES  # batches per core

F32 = mybir.dt.float32
BF16 = mybir.dt.bfloat16
F8 = mybir.dt.float8e4
ALU = mybir.AluOpType
ACTF = mybir.ActivationFunctionType
DR = mybir.MatmulPerfMode.DoubleRow

SB = S // 128   # 8 s-blocks per batch
HB = H // 128   # 8 h-blocks
OCH = H // 512  # 2 o-chunks (PSUM bank width)

SE = 16.0       # enc fp8 scale
SW = 64.0       # W_h fp8 scale
SFOLD = 32.0    # fold lhs-ones / rhs-dec scale (SFOLD^2 == SE*SW)
INV = 1.0 / (SE * SW)


def _build_kernel(tc, aps):
    nc = tc.nc
    enc, sth, mask, cov, wh, ws, bs, wc, v = (
        aps["encoder_outputs"], aps["s_t_hat"], aps["enc_padding_mask"],
        aps["coverage"], aps["W_h"], aps["W_s"], aps["b_s"], aps["W_c"], aps["v"],
    )
    ct_o, at_o, cn_o = aps["ct_out"], aps["attn_out"], aps["covnew_out"]

    id_dram = nc.inline_tensor(np.eye(128, dtype=ml_dtypes.bfloat16), name="id128")
    idf_dram = nc.inline_tensor(np.eye(128, dtype=np.float32), name="id128f")
    dec_dram = nc.dram_tensor("dec_bounce", [BL, H], F8).ap()

    from contextlib import ExitStack
    ctx = ExitStack()
    with ctx:
        # ---------------- pools ----------------
        consts = ctx.enter_context(tc.tile_pool(name="consts", bufs=1))
        wpool = ctx.enter_context(tc.tile_pool(name="wpool", bufs=1))
        natf = ctx.enter_context(tc.tile_pool(name="natf", bufs=10))   # f32 staging
        # nat tiles [128, H] bf16; 3 parity pools so a slot is reused only
        # 3 batches later (avoids WAR chains to the in-flight batch).
        natps = [ctx.enter_context(tc.tile_pool(name=f"natp{i}", bufs=8))
                 for i in range(3)]
        encbfps = [ctx.enter_context(tc.tile_pool(name=f"encbfp{i}", bufs=4))
                   for i in range(2)]                       # [128,HB,128] bf16
        enc8ps = [ctx.enter_context(tc.tile_pool(name=f"enc8p{i}", bufs=8))
                  for i in range(2)]                        # [128,HB,128] fp8
        ep = ctx.enter_context(tc.tile_pool(name="ep", bufs=3))       # e bf16
        vscrp = ctx.enter_context(tc.tile_pool(name="vscrp", bufs=2))
        smp = ctx.enter_context(tc.tile_pool(name="smp", bufs=2))

        # ---------------- constants ----------------
        id_bf = consts.tile([128, 128], BF16, tag="id")
        nc.sync.dma_start(id_bf[:], id_dram.ap())
        id_f = consts.tile([128, 128], F32, tag="idf")
        nc.sync.dma_start(id_f[:], idf_dram.ap())
        ones_1x128_bf = consts.tile([1, 128], BF16, tag="o1x128b")
        nc.vector.memset(ones_1x128_bf[:], 1.0)
        ones_1x128_f = consts.tile([1, 128], F32, tag="o1x128f")
        nc.vector.memset(ones_1x128_f[:], 1.0)
        ones_1x8_bf = consts.tile([1, 8], BF16, tag="o1x8b")
        nc.vector.memset(ones_1x8_bf[:], 1.0)
        ones_col_f = consts.tile([128, 1], F32, tag="ocolf")
        nc.vector.memset(ones_col_f[:], 1.0)

        enc_f32 = [[None] * SB for _ in range(BL)]
        enc_nat = [[None] * SB for _ in range(BL)]
        encbf_t = [[None] * SB for _ in range(BL)]
        enc8_t = [[None] * SB for _ in range(BL)]

        def load_f32(b, sb):
            """HWDGE f32 load of one enc s-block, alternating queues."""
            t = natf.tile([128, H], F32, tag="natf", name=f"ef{b}_{sb}")
            enc_f32[b][sb] = t
            eng = nc.sync if sb % 2 == 0 else nc.scalar
            eng.dma_start(t[:], enc[b, sb * 128:(sb + 1) * 128, :])

        def cast_nat(b, sb):
            """f32 -> bf16, alternating DVE / GpSimd-Pool."""
            nat = natps[b % 3].tile([128, H], BF16, tag="nat", name=f"en{b}_{sb}")
            enc_nat[b][sb] = nat
            eng = nc.vector if sb % 2 == 0 else nc.scalar
            if eng is nc.vector:
                eng.tensor_copy(nat[:], enc_f32[b][sb][:])
            else:
                nc.scalar.copy(nat[:], enc_f32[b][sb][:])

        def make_encT_sb(b, sb):
            """xbar transpose one s-block -> [h_p, hb, s128] bf16."""
            encT = encbfps[b % 2].tile([128, HB, 128], BF16, tag="encbf",
                                       name=f"eT{b}_{sb}")
            encbf_t[b][sb] = encT
            nc.sync.dma_start(encT[:], enc_nat[b][sb][:], transpose=True)

        def cast_encT_sb(b, sb):
            e8 = enc8ps[b % 2].tile([128, HB, 128], F8, tag="enc8",
                                    name=f"e8{b}_{sb}")
            enc8_t[b][sb] = e8
            nc.vector.tensor_scalar_mul(e8[:], encbf_t[b][sb][:], SE)

        # ---- startup ----
        # tiny rows via SWDGE (gpsimd queue is otherwise idle at t=0)
        bs_row = consts.tile([1, H], BF16, tag="bsrow")
        nc.gpsimd.dma_start(bs_row[:], bs[:])
        v_row = consts.tile([1, H], BF16, tag="vrow")
        nc.gpsimd.dma_start(v_row[:], v[:, :])

        # DMA triggers, ordered so the critical chain (b0 enc, W_s->dec,
        # W_h, then b1) streams in as it is consumed.
        s_f32 = natf.tile([BL, H], F32, tag="natf", name="sf32")
        nc.scalar.dma_start(s_f32[:], sth[:, :])
        # mask/cov: contiguous row loads (a strided "(j p) -> p j" DMA
        # scatters into 4-byte packets and jams the DMA engines); column
        # layout is built via PE transposes below.
        m_rows = natf.tile([BL, S], F32, tag="natf", name="mrows")
        nc.scalar.dma_start(m_rows[:], mask[:, :])
        c_rows = natf.tile([BL, S], F32, tag="natf", name="crows")
        nc.scalar.dma_start(c_rows[:], cov[:, :])
        for sb in range(SB):
            load_f32(0, sb)
        ws_f = [natf.tile([128, H], F32, tag="natf", name=f"wsf{i}")
                for i in range(HB)]
        for ob in range(HB):
            nc.scalar.dma_start(ws_f[ob][:], ws[ob * 128:(ob + 1) * 128, :])
        wh_f = [natf.tile([128, H], F32, tag="natf", name=f"whf{i}")
                for i in range(HB)]
        for ob in range(HB):
            nc.sync.dma_start(wh_f[ob][:], wh[ob * 128:(ob + 1) * 128, :])
        wc_f32 = consts.tile([1, H], F32, tag="wcf32")
        nc.sync.dma_start(wc_f32[:], wc[:, :])

        # b0 bf16 casts, then xbar transposes (sync queue, after wh
        # triggers), then fp8 casts interleaved with W bf16 casts on DVE
        s_bf = consts.tile([BL, H], BF16, tag="sbf")
        nc.vector.tensor_copy(s_bf[:], s_f32[:])
        for sb in range(SB):
            cast_nat(0, sb)
        for sb in range(SB):
            make_encT_sb(0, sb)
        wbfp_cm = tc.tile_pool(name="wbfp", bufs=8)
        wbfp = wbfp_cm.__enter__()
        ws_bf = [None] * HB
        wh_bf = [None] * HB
        for ob in range(HB):
            ws_bf[ob] = wbfp.tile([128, H], BF16, tag="wbf", name=f"wsb{ob}")
            nc.vector.tensor_copy(ws_bf[ob][:], ws_f[ob][:])
            cast_encT_sb(0, ob)
        for ob in range(HB):
            wh_bf[ob] = wbfp.tile([128, H], BF16, tag="wbf", name=f"whb{ob}")
            nc.vector.tensor_copy(wh_bf[ob][:], wh_f[ob][:])
        for sb in range(SB):
            load_f32(1, sb)

        mask_col = consts.tile([128, BL, SB], F32, tag="mcol")
        cov_col = consts.tile([128, BL, SB], F32, tag="ccol")

        # wc8 = W_c * (SE*SW) in fp8
        wc8_row = consts.tile([1, H], F8, tag="wc8row")
        nc.scalar.activation(wc8_row[:], wc_f32[:], ACTF.Copy, scale=SE * SW)

        psum_w_cm = tc.tile_pool(name="psum_w", bufs=1, space="PSUM")
        psum_t = psum_w_cm.__enter__()
        sT = consts.tile([128, HB, BL], BF16, tag="sT")
        for hb in range(HB):
            ptr = psum_t.tile([128, BL], BF16, tag="str")
            nc.tensor.transpose(ptr[:], s_bf[:, hb * 128:(hb + 1) * 128],
                                id_bf[0:BL, 0:BL])
            nc.scalar.copy(sT[:, hb, :], ptr[:])

        # mask/cov -> column layout [s%128, b, s//128] via PE transposes
        for j in range(SB):
            jsl = slice(j * 128, (j + 1) * 128)
            mt = psum_t.tile([128, BL], F32, tag="mtr")
            nc.tensor.transpose(mt[:], m_rows[:, jsl], id_f[0:BL, 0:BL])
            nc.scalar.copy(mask_col[:, :, j], mt[:])
            ct_ = psum_t.tile([128, BL], F32, tag="mtr")
            nc.tensor.transpose(ct_[:], c_rows[:, jsl], id_f[0:BL, 0:BL])
            nc.scalar.copy(cov_col[:, :, j], ct_[:])

        with tc.tile_pool(name="wsTp", bufs=1) as wsTp, \
             tc.tile_pool(name="psum_pro", bufs=1, space="PSUM") as psum_pro:
            # W_s: bf16 PE transpose, copy-out on ScalarE
            wsT = wsTp.tile([128, HB, H], BF16, tag="wsT")
            for ob in range(HB):
                ptr = psum_t.tile([128, HB, 128], BF16, tag="wtr")
                for hb in range(HB):
                    nc.tensor.transpose(ptr[:, hb, :],
                                        ws_bf[ob][:, hb * 128:(hb + 1) * 128],
                                        id_bf[:])
                nc.scalar.copy(wsT[:, :, ob * 128:(ob + 1) * 128], ptr[:])

            # dec_fea[b, o] = s_t_hat @ W_s.T + b_s  (PSUM partition = b)
            dec_ps = psum_pro.tile([BL, H], F32, tag="dec")
            for och in range(OCH):
                osl = slice(och * 512, (och + 1) * 512)
                for hb in range(HB):
                    nc.tensor.matmul(
                        dec_ps[:, osl], sT[:, hb, :], wsT[:, hb, osl],
                        start=(hb == 0), stop=False)
                nc.tensor.matmul(
                    dec_ps[:, osl], ones_1x8_bf[:], bs_row[:, osl],
                    start=False, stop=True)
            dec8_sb = consts.tile([BL, H], F8, tag="dec8sb")
            nc.scalar.activation(dec8_sb[:], dec_ps[:], ACTF.Copy, scale=SFOLD)
            nc.scalar.dma_start(dec_dram[:, :], dec8_sb[:])

            # W_h: bf16 PE transpose, fused *SW fp8 cast on copy-out (DVE)
            wh8T = wpool.tile([128, HB, H], F8, tag="wh8T")   # [h_p, hb, o]
            for ob in range(HB):
                ptr = psum_t.tile([128, HB, 128], BF16, tag="wtr")
                for hb in range(HB):
                    nc.tensor.transpose(ptr[:, hb, :],
                                        wh_bf[ob][:, hb * 128:(hb + 1) * 128],
                                        id_bf[:])
                nc.vector.tensor_scalar_mul(wh8T[:, :, ob * 128:(ob + 1) * 128],
                                            ptr[:], SW)

            # v broadcast to all 128 partitions (via ones outer-product)
            vb_ps = psum_pro.tile([128, 512], F32, tag="vb")
            v_bcast = consts.tile([128, H], BF16, tag="vbc")
            for och in range(OCH):
                osl = slice(och * 512, (och + 1) * 512)
                nc.tensor.matmul(vb_ps[:], ones_1x128_bf[:], v_row[:, osl],
                                 start=True, stop=True)
                nc.scalar.copy(v_bcast[:, osl], vb_ps[:])

        psum_w_cm.__exit__(None, None, None)
        wbfp_cm.__exit__(None, None, None)

        # fold tiles (2-parity, persistent): lhs [2,2,S]: sub0 = [32*ones; cov],
        # sub1 = 0.  rhs [2,2,H]: sub0 = [dec*32; wc*1024], sub1 = 0.
        fl8 = [consts.tile([2, 2, S], F8, tag=f"fl8_{i}", name=f"fl8_{i}")
               for i in range(2)]
        fr8 = [consts.tile([2, 2, H], F8, tag=f"fr8_{i}", name=f"fr8_{i}")
               for i in range(2)]
        for i in range(2):
            nc.vector.memset(fl8[i][:], 0.0)
            nc.vector.memset(fl8[i][0:1, 0:1, :], SFOLD)
            nc.vector.memset(fr8[i][:], 0.0)
            nc.scalar.dma_start(fr8[i][1:2, 0:1, :], wc8_row[:])

        def build_fold(b):
            nc.gpsimd.dma_start(fl8[b % 2][1:2, 0:1, :], cov[b, :])
            nc.scalar.dma_start(fr8[b % 2][0:1, 0:1, :], dec_dram[b, :])

        build_fold(0)

        # ---------------- main loop ----------------
        psum_e = ctx.enter_context(tc.tile_pool(name="psum_e", bufs=3, space="PSUM"))
        psum_ct = ctx.enter_context(tc.tile_pool(name="psum_ct", bufs=4, space="PSUM"))
        psum_z = ctx.enter_context(tc.tile_pool(name="psum_z", bufs=1, space="PSUM"))

        def finish_batch(b, wm, w_bf, ct_ps):
            """Z, 1/Z, attn/covnew/ct outputs for batch b.  Emitted inside
            batch b+1's stream so the PE never waits on the DVE chain."""
            rowsum = smp.tile([128, 1], F32, tag="rowsum")
            nc.vector.tensor_reduce(rowsum[:], wm[:], mybir.AxisListType.X, ALU.add)
            zps = psum_z.tile([1, 1], F32, tag="z")
            nc.tensor.matmul(zps[:], ones_col_f[:], rowsum[:], start=True, stop=True)
            z_sb = smp.tile([1, 1], F32, tag="zsb")
            nc.vector.tensor_copy(z_sb[:], zps[:])
            zb_ps = psum_z.tile([128, 1], F32, tag="z")
            nc.tensor.matmul(zb_ps[:], ones_1x128_f[:], z_sb[:], start=True, stop=True)
            zb = smp.tile([128, 1], F32, tag="zb")
            nc.vector.tensor_copy(zb[:], zb_ps[:])
            rz = smp.tile([128, 1], F32, tag="rz")
            nc.vector.reciprocal(rz[:], zb[:])

            attn_c = smp.tile([128, SB], F32, tag="attnc")
            nc.vector.tensor_scalar_mul(attn_c[:], wm[:], rz[:, 0:1])
            covn_c = smp.tile([128, SB], F32, tag="covnc")
            nc.vector.tensor_tensor(covn_c[:], attn_c[:], cov_col[:, b, :], ALU.add)
            # transpose [s128, j] -> [j, s128] so the output DMA is
            # contiguous (512B runs instead of 4B scatter packets)
            atp = psum_z.tile([BL, 128], F32, tag="z")
            nc.tensor.transpose(atp[:], attn_c[:], id_f[:])
            at_row = smp.tile([BL, 128], F32, tag="atrow")
            nc.vector.tensor_copy(at_row[:], atp[:])
            nc.gpsimd.dma_start(at_o[b, :].rearrange("(j p) -> j p", p=128),
                                at_row[:])
            cvp = psum_z.tile([BL, 128], F32, tag="z")
            nc.tensor.transpose(cvp[:], covn_c[:], id_f[:])
            cv_row = smp.tile([BL, 128], F32, tag="cvrow")
            nc.vector.tensor_copy(cv_row[:], cvp[:])
            nc.gpsimd.dma_start(cn_o[b, :].rearrange("(j p) -> j p", p=128),
                                cv_row[:])

            ct_sb = smp.tile([1, H], F32, tag="ctsb")
            for och in range(OCH):
                nc.vector.tensor_scalar_mul(
                    ct_sb[:, och * 512:(och + 1) * 512], ct_ps[och][:], rz[0:1, 0:1])
            nc.gpsimd.dma_start(ct_o[b, :], ct_sb[:])

        prev_fin = None
        for b in range(BL):
            if b + 1 < BL:
                build_fold(b + 1)

            sc = smp.tile([128, SB], F32, tag="scores")
            we = smp.tile([128, SB], F32, tag="we")
            wm = smp.tile([128, SB], F32, tag="wm")
            w_bf = smp.tile([128, SB], BF16, tag="wbf")
            ct_ps = [psum_ct.tile([1, 512], F32, tag="ct", name=f"ctps{och}")
                     for och in range(OCH)]

            def ct_mm(sb, b=b, w_bf=w_bf, ct_ps=ct_ps):
                for och in range(OCH):
                    nc.tensor.matmul(
                        ct_ps[och][:], w_bf[:, sb:sb + 1],
                        enc_nat[b][sb][:, och * 512:(och + 1) * 512],
                        start=(sb == 0), stop=(sb == SB - 1))

            for sb in range(SB):
                # pipelined ingest: load b+2, cast b+1, xbar b+1 (lag 1),
                # fp8 b+1 (lag 2)
                if b + 2 < BL:
                    load_f32(b + 2, sb)
                if b + 1 < BL:
                    cast_nat(b + 1, sb)
                    if sb >= 1:
                        make_encT_sb(b + 1, sb - 1)
                    if sb >= 2:
                        cast_encT_sb(b + 1, sb - 2)
                ssl = slice(sb * 128, (sb + 1) * 128)
                spart = smp.tile([128, OCH], F32, tag="spart")
                for och in range(OCH):
                    osl = slice(och * 512, (och + 1) * 512)
                    pe = psum_e.tile([128, 512], F32, tag="pe")
                    for k in range(HB // 2):
                        nc.tensor.matmul(pe[:], enc8_t[b][sb][:, 2 * k:2 * k + 2, :],
                                         wh8T[:, 2 * k:2 * k + 2, osl],
                                         start=(k == 0), stop=False, perf_mode=DR)
                    nc.tensor.matmul(pe[:], fl8[b % 2][:, :, ssl],
                                     fr8[b % 2][:, :, osl],
                                     start=False, stop=True, perf_mode=DR)
                    e_bf = ep.tile([128, 512], BF16, tag="e")
                    nc.scalar.activation(e_bf[:], pe[:], ACTF.Tanh, scale=INV)
                    vscr = vscrp.tile([128, 512], BF16, tag="vscr")
                    nc.vector.scalar_tensor_tensor(
                        out=vscr[:], in0=e_bf[:], scalar=1.0,
                        in1=v_bcast[:, osl], op0=ALU.mult, op1=ALU.mult,
                        accum_out=spart[:, och:och + 1])
                nc.vector.tensor_tensor(sc[:, sb:sb + 1], spart[:, 0:1],
                                        spart[:, 1:2], ALU.add)
                nc.scalar.activation(we[:, sb:sb + 1], sc[:, sb:sb + 1], ACTF.Exp)
                nc.vector.tensor_tensor(wm[:, sb:sb + 1], we[:, sb:sb + 1],
                                        mask_col[:, b, sb:sb + 1], ALU.mult)
                nc.vector.tensor_copy(w_bf[:, sb:sb + 1], wm[:, sb:sb + 1])
                if sb > 0:
                    ct_mm(sb - 1)
                if sb == 0 and prev_fin is not None:
                    prev_fin()
            if b + 1 < BL:
                make_encT_sb(b + 1, SB - 1)
                cast_encT_sb(b + 1, SB - 2)
                cast_encT_sb(b + 1, SB - 1)
            ct_mm(SB - 1)
            prev_fin = (lambda b=b, wm=wm, w_bf=w_bf, ct_ps=ct_ps:
                        finish_batch(b, wm, w_bf, ct_ps))
        prev_fin()


def build():
    nc = bacc.Bacc("TRN2", target_bir_lowering=False, debug=False,
                   num_devices=N_CORES)
    aps = {}
    aps["encoder_outputs"] = nc.dram_tensor(
        "encoder_outputs", [BL, S, H], F32, kind="ExternalInput").ap()
    aps["s_t_hat"] = nc.dram_tensor("s_t_hat", [BL, H], F32, kind="ExternalInput").ap()
    aps["enc_padding_mask"] = nc.dram_tensor(
        "enc_padding_mask", [BL, S], F32, kind="ExternalInput").ap()
    aps["coverage"] = nc.dram_tensor("coverage", [BL, S], F32, kind="ExternalInput").ap()
    aps["W_h"] = nc.dram_tensor("W_h", [H, H], F32, kind="ExternalInput").ap()
    aps["W_s"] = nc.dram_tensor("W_s", [H, H], F32, kind="ExternalInput").ap()
    aps["b_s"] = nc.dram_tensor("b_s", [H], F32, kind="ExternalInput").ap()
    aps["W_c"] = nc.dram_tensor("W_c", [H, 1], F32, kind="ExternalInput").ap()
    aps["v"] = nc.dram_tensor("v", [1, H], F32, kind="ExternalInput").ap()
    aps["ct_out"] = nc.dram_tensor("ct_out", [BL, H], F32, kind="ExternalOutput").ap()
    aps["attn_out"] = nc.dram_tensor("attn_out", [BL, S], F32, kind="ExternalOutput").ap()
    aps["covnew_out"] = nc.dram_tensor("covnew_out", [BL, S], F32, kind="ExternalOutput").ap()

    with tile.TileContext(nc) as tc:
        _build_kernel(tc, aps)
    nc.compile()
    return nc


_NC_CACHE = {}


def _get_nc():
    if "nc" not in _NC_CACHE:
        _NC_CACHE["nc"] = build()
    return _NC_CACHE["nc"]


def kernel(s_t_hat, encoder_outputs, enc_padding_mask, coverage,
           W_h, W_s, b_s, W_c, v, _trace=False, _tmpdir=None):
    f = lambda x: np.ascontiguousarray(np.asarray(x), dtype=np.float32)
    s_t_hat, encoder_outputs = f(s_t_hat), f(encoder_outputs)
    enc_padding_mask, coverage = f(enc_padding_mask), f(coverage)
    W_h, W_s, b_s, W_c, v = f(W_h), f(W_s), f(b_s), f(W_c), f(v)

    nc = _get_nc()
    in_maps = []
    for i in range(N_CORES):
        sl = slice(i * BL, (i + 1) * BL)
        in_maps.append({
            "encoder_outputs": encoder_outputs[sl],
            "s_t_hat": s_t_hat[sl],
            "enc_padding_mask": enc_padding_mask[sl],
            "coverage": coverage[sl],
            "W_h": W_h, "W_s": W_s, "b_s": b_s, "W_c": W_c, "v": v,
        })
    res = run_bass_kernel_spmd(nc, in_maps, core_ids=list(range(N_CORES)),
                               trace=_trace, tmpdir=_tmpdir)
    ct = np.concatenate([res.results[i]["ct_out"] for i in range(N_CORES)], axis=0)
    at = np.concatenate([res.results[i]["attn_out"] for i in range(N_CORES)], axis=0)
    cn = np.concatenate([res.results[i]["covnew_out"] for i in range(N_CORES)], axis=0)
    kernel._last_results = res
    return ct, at, cn


# revision 16
# speedup vs baseline: 1.0393x; 1.0393x over previous
"""Trainium2 Bass kernel for the coverage-attention module (fp8 DoubleRow).

Math (per batch b):
    enc_feat = encoder_outputs @ W_h.T                      [S, H]
    dec_fea  = s_t_hat @ W_s.T + b_s                        [H]
    e        = tanh(enc_feat + dec_fea + coverage[:,None]*W_c[:,0])
    scores   = e @ v[0]                                     [S]
    w        = exp(scores) * mask          (softmax+mask+renorm == w/sum(w))
    attn     = w / sum(w)
    c_t      = attn @ encoder_outputs                       [H]
    coverage_new = coverage + attn

Distribution: pure data-parallel over batch, 8 batches per NeuronCore,
weights replicated.  No collectives.

Per-core dataflow:
  - enc ingest: f32 HWDGE loads split across the sync+scalar queues
    (the SWDGE cast path is ~6.6us per tile and would starve the PE),
    f32->bf16 cast split across DVE and GpSimd-Pool, xbar DMA-transpose
    to [h_p, hb, s] bf16, DVE cast *16 -> fp8.  The chain runs with a
    1-2 s-block lag inside the previous batch so no queue ever stalls.
  - W_h/W_s: f32 loads, f32 PE transposes, fused fp8*64 (resp. bf16)
    cast on the PSUM->SBUF copy.
  - main matmul in fp8 DoubleRow (2 h-blocks per instruction, 2x PE
    throughput); PSUM holds 1024*enc_feat, tanh applies scale 1/1024.
  - dec_fea + coverage*W_c folded into each PSUM group as one extra
    [2,2]-DoubleRow matmul (operands pre-scaled: ones*32, dec*32,
    cov*1, W_c*1024).
  - scores via VectorE scalar_tensor_tensor against broadcast v.
  - softmax runs incrementally per s-block: exp/mask/cast as soon as a
    block's scores exist, and c_t accumulates in PSUM (k=s matmuls on
    the bf16 nat tiles) interleaved with the next block's main matmuls.
    Only the 1/Z normalization happens at batch end -> tiny tail.
"""

import numpy as np
import ml_dtypes

import concourse.bass as bass
import concourse.tile as tile
from concourse import bacc, mybir
from concourse.bass_utils import run_bass_kernel_spmd

N_CORES = 8
B, S, H = 64, 1024, 1024
BL = B // N_CORES  # batches per core

F32 = mybir.dt.float32
BF16 = mybir.dt.bfloat16
F8 = mybir.dt.float8e4
ALU = mybir.AluOpType
ACTF = mybir.ActivationFunctionType
DR = mybir.MatmulPerfMode.DoubleRow

SB = S // 128   # 8 s-blocks per batch
HB = H // 128   # 8 h-blocks
OCH = H // 512  # 2 o-chunks (PSUM bank width)

SE = 16.0       # enc fp8 scale
SW = 64.0       # W_h fp8 scale
SFOLD = 32.0    # fold lhs-ones / rhs-dec scale (SFOLD^2 == SE*SW)
INV = 1.0 / (SE * SW)


def _build_kernel(tc, aps):
    nc = tc.nc
    enc, sth, mask, cov, wh, ws, bs, wc, v = (
        aps["encoder_outputs"], aps["s_t_hat"], aps["enc_padding_mask"],
        aps["coverage"], aps["W_h"], aps["W_s"], aps["b_s"], aps["W_c"], aps["v"],
    )
    ct_o, at_o, cn_o = aps["ct_out"], aps["attn_out"], aps["covnew_out"]

    id_dram = nc.inline_tensor(np.eye(128, dtype=ml_dtypes.bfloat16), name="id128")
    idf_dram = nc.inline_tensor(np.eye(128, dtype=np.float32), name="id128f")
    dec_dram = nc.dram_tensor("dec_bounce", [BL, H], F8).ap()

    from contextlib import ExitStack
    ctx = ExitStack()
    with ctx:
        # ---------------- pools ----------------
        consts = ctx.enter_context(tc.tile_pool(name="consts", bufs=1))
        wpool = ctx.enter_context(tc.tile_pool(name="wpool", bufs=1))
        natf = ctx.enter_context(tc.tile_pool(name="natf", bufs=10))   # f32 staging
        # nat tiles [128, H] bf16; 3 parity pools so a slot is reused only
        # 3 batches later (avoids WAR chains to the in-flight batch).
        natps = [ctx.enter_context(tc.tile_pool(name=f"natp{i}", bufs=8))
                 for i in range(3)]
        encbfps = [ctx.enter_context(tc.tile_pool(name=f"encbfp{i}", bufs=4))
                   for i in range(2)]                       # [128,HB,128] bf16
        enc8ps = [ctx.enter_context(tc.tile_pool(name=f"enc8p{i}", bufs=8))
                  for i in range(2)]                        # [128,HB,128] fp8
        ep = ctx.enter_context(tc.tile_pool(name="ep", bufs=3))       # e bf16
        vscrp = ctx.enter_context(tc.tile_pool(name="vscrp", bufs=2))
        smp = ctx.enter_context(tc.tile_pool(name="smp", bufs=2))

        # ---------------- constants ----------------
        id_bf = consts.tile([128, 128], BF16, tag="id")
        nc.sync.dma_start(id_bf[:], id_dram.ap())
        id_f = consts.tile([128, 128], F32, tag="idf")
        nc.sync.dma_start(id_f[:], idf_dram.ap())
        ones_1x128_bf = consts.tile([1, 128], BF16, tag="o1x128b")
        nc.vector.memset(ones_1x128_bf[:], 1.0)
        ones_1x128_f = consts.tile([1, 128], F32, tag="o1x128f")
        nc.vector.memset(ones_1x128_f[:], 1.0)
        ones_1x8_bf = consts.tile([1, 8], BF16, tag="o1x8b")
        nc.vector.memset(ones_1x8_bf[:], 1.0)
        ones_col_f = consts.tile([128, 1], F32, tag="ocolf")
        nc.vector.memset(ones_col_f[:], 1.0)

        enc_f32 = [[None] * SB for _ in range(BL)]
        enc_nat = [[None] * SB for _ in range(BL)]
        encbf_t = [[None] * SB for _ in range(BL)]
        enc8_t = [[None] * SB for _ in range(BL)]

        def load_f32(b, sb):
            """HWDGE f32 load of one enc s-block, alternating queues."""
            t = natf.tile([128, H], F32, tag="natf", name=f"ef{b}_{sb}")
            enc_f32[b][sb] = t
            eng = nc.sync if sb % 2 == 0 else nc.scalar
            eng.dma_start(t[:], enc[b, sb * 128:(sb + 1) * 128, :])

        def cast_nat(b, sb):
            """f32 -> bf16, alternating DVE / GpSimd-Pool."""
            nat = natps[b % 3].tile([128, H], BF16, tag="nat", name=f"en{b}_{sb}")
            enc_nat[b][sb] = nat
            eng = nc.vector if sb % 2 == 0 else nc.scalar
            if eng is nc.vector:
                eng.tensor_copy(nat[:], enc_f32[b][sb][:])
            else:
                nc.scalar.copy(nat[:], enc_f32[b][sb][:])

        def make_encT_sb(b, sb):
            """xbar transpose one s-block -> [h_p, hb, s128] bf16."""
            encT = encbfps[b % 2].tile([128, HB, 128], BF16, tag="encbf",
                                       name=f"eT{b}_{sb}")
            encbf_t[b][sb] = encT
            nc.sync.dma_start(encT[:], enc_nat[b][sb][:], transpose=True)

        def cast_encT_sb(b, sb):
            e8 = enc8ps[b % 2].tile([128, HB, 128], F8, tag="enc8",
                                    name=f"e8{b}_{sb}")
            enc8_t[b][sb] = e8
            nc.vector.tensor_scalar_mul(e8[:], encbf_t[b][sb][:], SE)

        # ---- startup ----
        # tiny rows via SWDGE (gpsimd queue is otherwise idle at t=0)
        bs_row = consts.tile([1, H], BF16, tag="bsrow")
        nc.gpsimd.dma_start(bs_row[:], bs[:])
        v_row = consts.tile([1, H], BF16, tag="vrow")
        nc.gpsimd.dma_start(v_row[:], v[:, :])

        # f32 loads: b0, then W_s (scalar) / W_h (sync), then b1
        s_f32 = natf.tile([BL, H], F32, tag="natf", name="sf32")
        nc.scalar.dma_start(s_f32[:], sth[:, :])
        for sb in range(SB):
            load_f32(0, sb)
        ws_f = [natf.tile([128, H], F32, tag="natf", name=f"wsf{i}")
                for i in range(HB)]
        for ob in range(HB):
            nc.scalar.dma_start(ws_f[ob][:], ws[ob * 128:(ob + 1) * 128, :])
        wh_f = [natf.tile([128, H], F32, tag="natf", name=f"whf{i}")
                for i in range(HB)]
        for ob in range(HB):
            nc.sync.dma_start(wh_f[ob][:], wh[ob * 128:(ob + 1) * 128, :])
        for sb in range(SB):
            load_f32(1, sb)
        wc_f32 = consts.tile([1, H], F32, tag="wcf32")
        nc.sync.dma_start(wc_f32[:], wc[:, :])

        # b0 bf16 casts + xbar transposes + fp8 casts
        for sb in range(SB):
            cast_nat(0, sb)
        for sb in range(SB):
            make_encT_sb(0, sb)
            cast_encT_sb(0, sb)

        # mask/cov: contiguous row loads (a strided "(j p) -> p j" DMA
        # scatters into 4-byte packets and jams the DMA engines); column
        # layout is built via PE transposes below.
        m_rows = natf.tile([BL, S], F32, tag="natf", name="mrows")
        nc.scalar.dma_start(m_rows[:], mask[:, :])
        c_rows = natf.tile([BL, S], F32, tag="natf", name="crows")
        nc.scalar.dma_start(c_rows[:], cov[:, :])
        mask_col = consts.tile([128, BL, SB], F32, tag="mcol")
        cov_col = consts.tile([128, BL, SB], F32, tag="ccol")

        # wc8 = W_c * (SE*SW) in fp8
        wc8_row = consts.tile([1, H], F8, tag="wc8row")
        nc.scalar.activation(wc8_row[:], wc_f32[:], ACTF.Copy, scale=SE * SW)

        # s_bf / sT
        psum_w_cm = tc.tile_pool(name="psum_w", bufs=1, space="PSUM")
        psum_t = psum_w_cm.__enter__()
        s_bf = consts.tile([BL, H], BF16, tag="sbf")
        nc.vector.tensor_copy(s_bf[:], s_f32[:])
        sT = consts.tile([128, HB, BL], BF16, tag="sT")
        for hb in range(HB):
            ptr = psum_t.tile([128, BL], BF16, tag="str")
            nc.tensor.transpose(ptr[:], s_bf[:, hb * 128:(hb + 1) * 128],
                                id_bf[0:BL, 0:BL])
            nc.scalar.copy(sT[:, hb, :], ptr[:])

        # mask/cov -> column layout [s%128, b, s//128] via PE transposes
        for j in range(SB):
            jsl = slice(j * 128, (j + 1) * 128)
            mt = psum_t.tile([128, BL], F32, tag="mtr")
            nc.tensor.transpose(mt[:], m_rows[:, jsl], id_f[0:BL, 0:BL])
            nc.scalar.copy(mask_col[:, :, j], mt[:])
            ct_ = psum_t.tile([128, BL], F32, tag="mtr")
            nc.tensor.transpose(ct_[:], c_rows[:, jsl], id_f[0:BL, 0:BL])
            nc.scalar.copy(cov_col[:, :, j], ct_[:])

        with tc.tile_pool(name="wsTp", bufs=1) as wsTp, \
             tc.tile_pool(name="psum_pro", bufs=1, space="PSUM") as psum_pro:
            # W_s: f32 PE transpose, bf16 on copy-out
            wsT = wsTp.tile([128, HB, H], BF16, tag="wsT")
            for ob in range(HB):
                ptr = psum_t.tile([128, HB, 128], F32, tag="wtr")
                for hb in range(HB):
                    nc.tensor.transpose(ptr[:, hb, :],
                                        ws_f[ob][:, hb * 128:(hb + 1) * 128],
                                        id_f[:])
                nc.scalar.copy(wsT[:, :, ob * 128:(ob + 1) * 128], ptr[:])

            # dec_fea[b, o] = s_t_hat @ W_s.T + b_s  (PSUM partition = b)
            dec_ps = psum_pro.tile([BL, H], F32, tag="dec")
            for och in range(OCH):
                osl = slice(och * 512, (och + 1) * 512)
                for hb in range(HB):
                    nc.tensor.matmul(
                        dec_ps[:, osl], sT[:, hb, :], wsT[:, hb, osl],
                        start=(hb == 0), stop=False)
                nc.tensor.matmul(
                    dec_ps[:, osl], ones_1x8_bf[:], bs_row[:, osl],
                    start=False, stop=True)
            dec8_sb = consts.tile([BL, H], F8, tag="dec8sb")
            nc.scalar.activation(dec8_sb[:], dec_ps[:], ACTF.Copy, scale=SFOLD)
            nc.scalar.dma_start(dec_dram[:, :], dec8_sb[:])

            # W_h: f32 PE transpose, fused *SW fp8 cast on copy-out (DVE)
            wh8T = wpool.tile([128, HB, H], F8, tag="wh8T")   # [h_p, hb, o]
            for ob in range(HB):
                ptr = psum_t.tile([128, HB, 128], F32, tag="wtr")
                for hb in range(HB):
                    nc.tensor.transpose(ptr[:, hb, :],
                                        wh_f[ob][:, hb * 128:(hb + 1) * 128],
                                        id_f[:])
                nc.vector.tensor_scalar_mul(wh8T[:, :, ob * 128:(ob + 1) * 128],
                                            ptr[:], SW)

            # v broadcast to all 128 partitions (via ones outer-product)
            vb_ps = psum_pro.tile([128, 512], F32, tag="vb")
            v_bcast = consts.tile([128, H], BF16, tag="vbc")
            for och in range(OCH):
                osl = slice(och * 512, (och + 1) * 512)
                nc.tensor.matmul(vb_ps[:], ones_1x128_bf[:], v_row[:, osl],
                                 start=True, stop=True)
                nc.scalar.copy(v_bcast[:, osl], vb_ps[:])

        psum_w_cm.__exit__(None, None, None)

        # fold tiles (2-parity, persistent): lhs [2,2,S]: sub0 = [32*ones; cov],
        # sub1 = 0.  rhs [2,2,H]: sub0 = [dec*32; wc*1024], sub1 = 0.
        fl8 = [consts.tile([2, 2, S], F8, tag=f"fl8_{i}", name=f"fl8_{i}")
               for i in range(2)]
        fr8 = [consts.tile([2, 2, H], F8, tag=f"fr8_{i}", name=f"fr8_{i}")
               for i in range(2)]
        for i in range(2):
            nc.vector.memset(fl8[i][:], 0.0)
            nc.vector.memset(fl8[i][0:1, 0:1, :], SFOLD)
            nc.vector.memset(fr8[i][:], 0.0)
            nc.scalar.dma_start(fr8[i][1:2, 0:1, :], wc8_row[:])

        def build_fold(b):
            nc.gpsimd.dma_start(fl8[b % 2][1:2, 0:1, :], cov[b, :])
            nc.scalar.dma_start(fr8[b % 2][0:1, 0:1, :], dec_dram[b, :])

        build_fold(0)

        # ---------------- main loop ----------------
        psum_e = ctx.enter_context(tc.tile_pool(name="psum_e", bufs=3, space="PSUM"))
        psum_ct = ctx.enter_context(tc.tile_pool(name="psum_ct", bufs=4, space="PSUM"))
        psum_z = ctx.enter_context(tc.tile_pool(name="psum_z", bufs=1, space="PSUM"))

        def finish_batch(b, wm, w_bf, ct_ps):
            """Z, 1/Z, attn/covnew/ct outputs for batch b.  Emitted inside
            batch b+1's stream so the PE never waits on the DVE chain."""
            rowsum = smp.tile([128, 1], F32, tag="rowsum")
            nc.vector.tensor_reduce(rowsum[:], wm[:], mybir.AxisListType.X, ALU.add)
            zps = psum_z.tile([1, 1], F32, tag="z")
            nc.tensor.matmul(zps[:], ones_col_f[:], rowsum[:], start=True, stop=True)
            z_sb = smp.tile([1, 1], F32, tag="zsb")
            nc.vector.tensor_copy(z_sb[:], zps[:])
            zb_ps = psum_z.tile([128, 1], F32, tag="z")
            nc.tensor.matmul(zb_ps[:], ones_1x128_f[:], z_sb[:], start=True, stop=True)
            zb = smp.tile([128, 1], F32, tag="zb")
            nc.vector.tensor_copy(zb[:], zb_ps[:])
            rz = smp.tile([128, 1], F32, tag="rz")
            nc.vector.reciprocal(rz[:], zb[:])

            attn_c = smp.tile([128, SB], F32, tag="attnc")
            nc.vector.tensor_scalar_mul(attn_c[:], wm[:], rz[:, 0:1])
            covn_c = smp.tile([128, SB], F32, tag="covnc")
            nc.vector.tensor_tensor(covn_c[:], attn_c[:], cov_col[:, b, :], ALU.add)
            # transpose [s128, j] -> [j, s128] so the output DMA is
            # contiguous (512B runs instead of 4B scatter packets)
            atp = psum_z.tile([BL, 128], F32, tag="z")
            nc.tensor.transpose(atp[:], attn_c[:], id_f[:])
            at_row = smp.tile([BL, 128], F32, tag="atrow")
            nc.vector.tensor_copy(at_row[:], atp[:])
            nc.gpsimd.dma_start(at_o[b, :].rearrange("(j p) -> j p", p=128),
                                at_row[:])
            cvp = psum_z.tile([BL, 128], F32, tag="z")
            nc.tensor.transpose(cvp[:], covn_c[:], id_f[:])
            cv_row = smp.tile([BL, 128], F32, tag="cvrow")
            nc.vector.tensor_copy(cv_row[:], cvp[:])
            nc.gpsimd.dma_start(cn_o[b, :].rearrange("(j p) -> j p", p=128),
                                cv_row[:])

            ct_sb = smp.tile([1, H], F32, tag="ctsb")
            for och in range(OCH):
                nc.vector.tensor_scalar_mul(
                    ct_sb[:, och * 512:(och + 1) * 512], ct_ps[och][:], rz[0:1, 0:1])
            nc.gpsimd.dma_start(ct_o[b, :], ct_sb[:])

        prev_fin = None
        for b in range(BL):
            if b + 1 < BL:
                build_fold(b + 1)

            sc = smp.tile([128, SB], F32, tag="scores")
            we = smp.tile([128, SB], F32, tag="we")
            wm = smp.tile([128, SB], F32, tag="wm")
            w_bf = smp.tile([128, SB], BF16, tag="wbf")
            ct_ps = [psum_ct.tile([1, 512], F32, tag="ct", name=f"ctps{och}")
                     for och in range(OCH)]

            def ct_mm(sb, b=b, w_bf=w_bf, ct_ps=ct_ps):
                for och in range(OCH):
                    nc.tensor.matmul(
                        ct_ps[och][:], w_bf[:, sb:sb + 1],
                        enc_nat[b][sb][:, och * 512:(och + 1) * 512],
                        start=(sb == 0), stop=(sb == SB - 1))

            for sb in range(SB):
                # pipelined ingest: load b+2, cast b+1, xbar b+1 (lag 1),
                # fp8 b+1 (lag 2)
                if b + 2 < BL:
                    load_f32(b + 2, sb)
                if b + 1 < BL:
                    cast_nat(b + 1, sb)
                    if sb >= 1:
                        make_encT_sb(b + 1, sb - 1)
                    if sb >= 2:
                        cast_encT_sb(b + 1, sb - 2)
                ssl = slice(sb * 128, (sb + 1) * 128)
                spart = smp.tile([128, OCH], F32, tag="spart")
                for och in range(OCH):
                    osl = slice(och * 512, (och + 1) * 512)
                    pe = psum_e.tile([128, 512], F32, tag="pe")
                    for k in range(HB // 2):
                        nc.tensor.matmul(pe[:], enc8_t[b][sb][:, 2 * k:2 * k + 2, :],
                                         wh8T[:, 2 * k:2 * k + 2, osl],
                                         start=(k == 0), stop=False, perf_mode=DR)
                    nc.tensor.matmul(pe[:], fl8[b % 2][:, :, ssl],
                                     fr8[b % 2][:, :, osl],
                                     start=False, stop=True, perf_mode=DR)
                    e_bf = ep.tile([128, 512], BF16, tag="e")
                    nc.scalar.activation(e_bf[:], pe[:], ACTF.Tanh, scale=INV)
                    vscr = vscrp.tile([128, 512], BF16, tag="vscr")
                    nc.vector.scalar_tensor_tensor(
                        out=vscr[:], in0=e_bf[:], scalar=1.0,
                        in1=v_bcast[:, osl], op0=ALU.mult, op1=ALU.mult,
                        accum_out=spart[:, och:och + 1])
                nc.vector.tensor_tensor(sc[:, sb:sb + 1], spart[:, 0:1],
                                        spart[:, 1:2], ALU.add)
                nc.scalar.activation(we[:, sb:sb + 1], sc[:, sb:sb + 1], ACTF.Exp)
                nc.vector.tensor_tensor(wm[:, sb:sb + 1], we[:, sb:sb + 1],
                                        mask_col[:, b, sb:sb + 1], ALU.mult)
                nc.vector.tensor_copy(w_bf[:, sb:sb + 1], wm[:, sb:sb + 1])
                if sb > 0:
                    ct_mm(sb - 1)
                if sb == 0 and prev_fin is not None:
                    prev_fin()
            if b + 1 < BL:
                make_encT_sb(b + 1, SB - 1)
                cast_encT_sb(b + 1, SB - 2)
                cast_encT_sb(b + 1, SB - 1)
            ct_mm(SB - 1)
            prev_fin = (lambda b=b, wm=wm, w_bf=w_bf, ct_ps=ct_ps:
                        finish_batch(b, wm, w_bf, ct_ps))
        prev_fin()


def build():
    nc = bacc.Bacc("TRN2", target_bir_lowering=False, debug=False,
                   num_devices=N_CORES)
    aps = {}
    aps["encoder_outputs"] = nc.dram_tensor(
        "encoder_outputs", [BL, S, H], F32, kind="ExternalInput").ap()
    aps["s_t_hat"] = nc.dram_tensor("s_t_hat", [BL, H], F32, kind="ExternalInput").ap()
    aps["enc_padding_mask"] = nc.dram_tensor(
        "enc_padding_mask", [BL, S], F32, kind="ExternalInput").ap()
    aps["coverage"] = nc.dram_tensor("coverage", [BL, S], F32, kind="ExternalInput").ap()
    aps["W_h"] = nc.dram_tensor("W_h", [H, H], F32, kind="ExternalInput").ap()
    aps["W_s"] = nc.dram_tensor("W_s", [H, H], F32, kind="ExternalInput").ap()
    aps["b_s"] = nc.dram_tensor("b_s", [H], F32, kind="ExternalInput").ap()
    aps["W_c"] = nc.dram_tensor("W_c", [H, 1], F32, kind="ExternalInput").ap()
    aps["v"] = nc.dram_tensor("v", [1, H], F32, kind="ExternalInput").ap()
    aps["ct_out"] = nc.dram_tensor("ct_out", [BL, H], F32, kind="ExternalOutput").ap()
    aps["attn_out"] = nc.dram_tensor("attn_out", [BL, S], F32, kind="ExternalOutput").ap()
    aps["covnew_out"] = nc.dram_tensor("covnew_out", [BL, S], F32, kind="ExternalOutput").ap()

    with tile.TileContext(nc) as tc:
        _build_kernel(tc, aps)
    nc.compile()
    return nc


_NC_CACHE = {}


def _get_nc():
    if "nc" not in _NC_CACHE:
        _NC_CACHE["nc"] = build()
    return _NC_CACHE["nc"]


def kernel(s_t_hat, encoder_outputs, enc_padding_mask, coverage,
           W_h, W_s, b_s, W_c, v, _trace=False, _tmpdir=None):
    f = lambda x: np.ascontiguousarray(np.asarray(x), dtype=np.float32)
    s_t_hat, encoder_outputs = f(s_t_hat), f(encoder_outputs)
    enc_padding_mask, coverage = f(enc_padding_mask), f(coverage)
    W_h, W_s, b_s, W_c, v = f(W_h), f(W_s), f(b_s), f(W_c), f(v)

    nc = _get_nc()
    in_maps = []
    for i in range(N_CORES):
        sl = slice(i * BL, (i + 1) * BL)
        in_maps.append({
            "encoder_outputs": encoder_outputs[sl],
            "s_t_hat": s_t_hat[sl],
            "enc_padding_mask": enc_padding_mask[sl],
            "coverage": coverage[sl],
            "W_h": W_h, "W_s": W_s, "b_s": b_s, "W_c": W_c, "v": v,
        })
    res = run_bass_kernel_spmd(nc, in_maps, core_ids=list(range(N_CORES)),
                               trace=_trace, tmpdir=_tmpdir)
    ct = np.concatenate([res.results[i]["ct_out"] for i in range(N_CORES)], axis=0)
    at = np.concatenate([res.results[i]["attn_out"] for i in range(N_CORES)], axis=0)
    cn = np.concatenate([res.results[i]["covnew_out"] for i in range(N_CORES)], axis=0)
    kernel._last_results = res
    return ct, at, cn
